# revision 10
# baseline (speedup 1.0000x reference)
"""Trainium2 Bass kernel for nn_AlgebraicFusion (complex bank mixer).

Algebra (per row l, bank n):
  y_n = P_n x_n            P_n = phase-folded bank_W (host precompute)
  w   = softmax(MLP(mean|x_n|^2))
  c   = sum_n w_n y_n
  out = O' (gamma * c * rsqrt(mean|c|^2 + eps))

v3 layout (per core, 1024 rows):
  - x arrives PRE-TRANSPOSED from host ([n, comp, icc, iw, l]) so chain
    moving operands come straight from DMA -- no PE transposes, no PSUM
    round-trip, no Scalar copies on the critical path.
  - router scales applied per-column: e_n broadcast over partitions with a
    K=1 matmul, then in-place DVE tensor_mul on the x^T tiles; xws = xr+xi.
  - 2 superblocks x 512 rows: BN=512 = one PSUM bank; 213ns matmuls fully
    hide 107ns LDWEIGHTS; pk streamed twice (50 MB).
  - chains: 4 "acc" PSUM banks; sb0-otp0 runs icc-outer (DMA arrival order)
    borrowing 2 "p5" banks so all three Karatsuba chains accumulate at once.
  - sb1's scale/xws work is interleaved into p5(sb0)'s DVE stream as filler
    ops so the DVE FIFO never blocks the superblock handoff.
  - p5 (out ComplexLinear, gamma+rsqrt folded) on 2 rotating PSUM banks.
  - mag/router from a separate row-major x stream (Scalar square-accum).
  - DMA queues: sync = pk + xt(n0,n1); gpsimd = xu + xt(n2,n3) + oks;
    scalar = out.

Sharding: rows (B*L = 8192) split evenly across 8 cores; weights replicated.
"""
import numpy as np

NB, B, L, D = 4, 4, 2048, 1024
EPS, NORM_EPS = 1e-8, 1e-5
NCORES = 8
ROWS = B * L // NCORES          # 1024 rows per core
NSB = 2                         # superblocks
SBT = 4                         # l-tiles per superblock
BN = SBT * 128                  # 512 rows per superblock
IC = 8                          # i-chunks (contraction 1024 = 8*128)
OT = 8                          # o-tiles
OTP = OT // 2                   # o-tile pairs
DH = 2                          # d2 halves of 512

XT_BUFS = 8
XWS_BUFS = 4
PKT_BUFS = 2
OKS_BUFS = 3
XU_BUFS = 3
CT_BUFS = 16
CTS_BUFS = 4


def build_program():
    import concourse.bacc as bacc
    import concourse.tile as tile
    import concourse.mybir as mybir

    AF = mybir.ActivationFunctionType
    from concourse.alu_op_type import AluOpType

    f16 = mybir.dt.float16
    f32 = mybir.dt.float32
    f8 = mybir.dt.float8e4

    nc = bacc.Bacc("TRN2", target_bir_lowering=False, debug=False,
                   num_devices=NCORES)

    xtd = nc.dram_tensor("xtd", [NB, 2, IC, 128, ROWS], f16,
                         kind="ExternalInput").ap()
    xud = nc.dram_tensor("xud", [NB, 2, ROWS, D], f16,
                         kind="ExternalInput").ap()
    pk = nc.dram_tensor("pk", [OTP, NB, 3, 128, 2048], f16,
                        kind="ExternalInput").ap()
    pk0d = nc.dram_tensor("pk0", [IC, NB, 3, 128, 256], f16,
                          kind="ExternalInput").ap()
    okk = nc.dram_tensor("okk", [DH, 3, 128, OT * 512], f16,
                         kind="ExternalInput").ap()
    w1t = nc.dram_tensor("w1t", [4, 16], f16, kind="ExternalInput").ap()
    b1c = nc.dram_tensor("b1c", [16, 1], f32, kind="ExternalInput").ap()
    w2t = nc.dram_tensor("w2t", [16, 4], f16, kind="ExternalInput").ap()
    b2c = nc.dram_tensor("b2c", [4, 1], f32, kind="ExternalInput").ap()
    eye16d = nc.dram_tensor("eye16", [128, 128], f16, kind="ExternalInput").ap()
    sel4d = nc.dram_tensor("sel4", [NB, NB * 128], f16,
                           kind="ExternalInput").ap()
    one1d = nc.dram_tensor("one1", [1, 1], f32, kind="ExternalInput").ap()
    outd = nc.dram_tensor("out", [ROWS, 2 * D], f32, kind="ExternalOutput").ap()

    with tile.TileContext(nc) as tc:
        import contextlib
        with contextlib.ExitStack() as ctx:
            cst = ctx.enter_context(tc.tile_pool(name="cst", bufs=1))
            wk = ctx.enter_context(tc.tile_pool(name="wk", bufs=1))
            ps = ctx.enter_context(tc.tile_pool(name="ps", bufs=1,
                                                space="PSUM"))

            eye16 = cst.tile([128, 128], f16)
            nc.sync.dma_start(eye16[:], eye16d)
            one1 = cst.tile([1, 1], f32)
            nc.sync.dma_start(one1[:], one1d)
            w1s = cst.tile([4, 16], f16)
            nc.sync.dma_start(w1s[:], w1t)
            b1s = cst.tile([16, 1], f32)
            nc.sync.dma_start(b1s[:], b1c)
            w2s = cst.tile([16, 4], f16)
            nc.sync.dma_start(w2s[:], w2t)
            b2s = cst.tile([4, 1], f32)
            nc.sync.dma_start(b2s[:], b2c)
            onesD = cst.tile([128, 1], f16)
            nc.vector.memset(onesD[:], 1.0 / D)
            ones4 = cst.tile([NB, 1], f16)
            nc.vector.memset(ones4[:], 1.0)
            sel4 = cst.tile([NB, NB * 128], f16)
            nc.sync.dma_start(sel4[:], sel4d)
            sseps = cst.tile([1, ROWS], f32)   # s^2 per row
            invrow = cst.tile([1, ROWS], f32)  # rsqrt factor per row

            st = {"xt": {}, "xws": {}, "wbn": {}, "ct": {}, "ctS": {},
                  "invc": {}, "e4sb": {}}

            def load_xu(t):
                """row-major x DMAs (scalar queue, lookahead-pipelined) +
                Scalar square-accum -> mag[t]."""
                magr = wk.tile([128, NB], f32, tag="magr", bufs=2,
                               name=f"magr{t}")
                magi = wk.tile([128, NB], f32, tag="magi", bufs=2,
                               name=f"magi{t}")
                pairs = [(n, comp) for n in range(NB) for comp in (0, 1)]
                xcs = {}

                def issue_dma(k):
                    n, comp = pairs[k]
                    xc = wk.tile([128, D], f16, tag="xu", bufs=XU_BUFS,
                                 name=f"xu{t}_{n}_{comp}")
                    nc.scalar.dma_start(
                        xc[:], xud[n, comp, t * 128:(t + 1) * 128, :])
                    xcs[k] = xc

                LOOK = 2
                for k in range(LOOK):
                    issue_dma(k)
                for k in range(len(pairs)):
                    if k + LOOK < len(pairs):
                        issue_dma(k + LOOK)
                    n, comp = pairs[k]
                    junk = wk.tile([128, D], f8, tag="junk", bufs=1,
                                   name=f"junk{t}_{n}_{comp}")
                    acc = magr if comp == 0 else magi
                    nc.scalar.activation(junk[:], xcs[k][:], AF.Square,
                                         accum_out=acc[:, n:n + 1])
                mag = wk.tile([128, NB], f16, tag="mag", bufs=2, name=f"mag{t}")
                nc.vector.tensor_add(mag[:], magr[:], magi[:])
                st[f"mag{t}"] = mag

            def router(t):
                """per-tile router MLP -> unnormalized scales into e4sb."""
                sb, tb = t // SBT, t % SBT
                if tb == 0:
                    st["e4sb"][sb] = wk.tile([NB, BN], f16, tag="e4sb",
                                             bufs=2, name=f"e4sb{sb}")
                e4sb = st["e4sb"][sb]
                mag = st.pop(f"mag{t}")
                mtp = ps.tile([NB, 128], f16, tag="sm", bufs=2, name=f"mtp{t}")
                nc.tensor.transpose(mtp[:], mag[:], eye16[:])
                magT = wk.tile([NB, 128], f16, tag="magT", bufs=2,
                               name=f"magT{t}")
                nc.vector.tensor_copy(magT[:], mtp[:])
                h1p = ps.tile([16, 128], f32, tag="sm", bufs=2, name=f"h1p{t}")
                nc.tensor.matmul(h1p[:], w1s[:], magT[:], start=True, stop=True)
                h1 = wk.tile([16, 128], f16, tag="h1", bufs=2, name=f"h1{t}")
                nc.scalar.activation(h1[:], h1p[:], AF.Gelu, bias=b1s[:])
                lgp = ps.tile([NB, 128], f32, tag="sm", bufs=2, name=f"lgp{t}")
                nc.tensor.matmul(lgp[:], w2s[:], h1[:], start=True, stop=True)
                nc.scalar.activation(e4sb[:, tb * 128:(tb + 1) * 128], lgp[:],
                                     AF.Exp, bias=b2s[:])
                ssump = ps.tile([1, 128], f32, tag="sm", bufs=2, name=f"ssump{t}")
                nc.tensor.matmul(ssump[:], ones4[:],
                                 e4sb[:, tb * 128:(tb + 1) * 128],
                                 start=True, stop=True)
                s_sb = wk.tile([1, 128], f32, tag="s_sb", bufs=2,
                               name=f"ssb{t}")
                nc.vector.tensor_copy(s_sb[:], ssump[:])
                nc.vector.tensor_mul(sseps[0:1, t * 128:(t + 1) * 128],
                                     s_sb[:], s_sb[:])

            def load_xt(sb, q_outer=False):
                """DMA pre-transposed x^T tiles (split across two queues).

                q_outer: issue DMAs i-chunk-pair-outer so arrival order
                matches the icc-outer chain consumption of sb0-otp0."""
                xt = {}
                for n in range(NB):
                    for comp in (0, 1):
                        xt[(n, comp)] = wk.tile(
                            [128, IC * BN], f16, tag="xt", bufs=XT_BUFS,
                            name=f"xt{sb}_{n}_{comp}")

                def issue(q, n, comp):
                    dst = xt[(n, comp)][:].rearrange(
                        "p (i l) -> p i l", i=IC)[:, 2 * q:2 * q + 2, :]
                    src = xtd[n, comp, 2 * q:2 * q + 2, :,
                              sb * BN:(sb + 1) * BN] \
                        .rearrange("i p l -> p i l")
                    eng = nc.sync if n < 2 else nc.gpsimd
                    eng.dma_start(dst, src)

                if q_outer:
                    for q in range(IC // 2):
                        for n in range(NB):
                            for comp in (0, 1):
                                issue(q, n, comp)
                else:
                    for n in range(NB):
                        for comp in (0, 1):
                            for q in range(IC // 2):
                                issue(q, n, comp)
                st["xt"][sb] = xt

            def wbn_prep(sb):
                """broadcast e_n over partitions into wbn tiles; alloc xws."""
                e4sb = st["e4sb"][sb]
                wbns = {}
                for n in range(NB):
                    wp = ps.tile([128, BN], f32, tag="sm", bufs=2, name=f"wp{sb}_{n}")
                    nc.tensor.matmul(wp[:], sel4[:, n * 128:(n + 1) * 128],
                                     e4sb[:], start=True, stop=True)
                    wbn = wk.tile([128, BN], f16, tag="wbn", bufs=4,
                                  name=f"wbn{sb}_{n}")
                    nc.vector.tensor_copy(wbn[:], wp[:])
                    wbns[n] = wbn
                st["wbn"][sb] = wbns
                st["xws"][sb] = {
                    n: wk.tile([128, IC * BN], f16, tag="xws", bufs=XWS_BUFS,
                               name=f"xws{sb}_{n}") for n in range(NB)}

            def scale_icc(sb, icc):
                """scale one i-chunk of all xt tiles in place + xws slices."""
                xt, xws, wbns = st["xt"][sb], st["xws"][sb], st["wbn"][sb]
                s0, s1 = icc * BN, (icc + 1) * BN
                for n in range(NB):
                    for comp in (0, 1):
                        sl = xt[(n, comp)][:, s0:s1]
                        nc.vector.tensor_mul(sl, sl, wbns[n][:])
                    nc.vector.tensor_add(xws[n][:, s0:s1],
                                         xt[(n, 0)][:, s0:s1],
                                         xt[(n, 1)][:, s0:s1])

            def scale_nc(sb, n, comp):
                """scale all i-chunks of one (n, comp) tile; xws on gpsimd."""
                xt, xws, wbns = st["xt"][sb], st["xws"][sb], st["wbn"][sb]
                for icc in range(IC):
                    sl = xt[(n, comp)][:, icc * BN:(icc + 1) * BN]
                    nc.vector.tensor_mul(sl, sl, wbns[n][:])
                if comp == 1:
                    nc.gpsimd.tensor_add(xws[n][:], xt[(n, 0)][:],
                                         xt[(n, 1)][:])

            def pk0_dma():
                """icc-major pk tiles for the DMA-paced first otp."""
                pkt0 = {}
                for icc in range(IC):
                    for n in range(NB):
                        for c in range(3):
                            pt = wk.tile([128, 256], f16, tag="pk0", bufs=8,
                                         name=f"pk0_{icc}_{n}_{c}")
                            nc.sync.dma_start(pt[:], pk0d[icc, n, c])
                            pkt0[(icc, n, c)] = pt
                return pkt0

            def oks_dma(sb, dh):
                oks = {}
                for okc in range(3):
                    ok = wk.tile([128, OT * 512], f16, tag="oks",
                                 bufs=OKS_BUFS, name=f"oks{sb}_{dh}_{okc}")
                    nc.gpsimd.dma_start(ok[:], okk[dh, okc])
                    oks[okc] = ok
                return oks

            def pk_dma(otp):
                pkt = {}
                for c in range(3):
                    for n in range(NB):
                        pt = wk.tile([128, 2048], f16, tag="pkt",
                                     bufs=PKT_BUFS, name=f"pkt{otp}_{c}_{n}")
                        nc.sync.dma_start(pt[:], pk[otp, n, c])
                        pkt[(c, n)] = pt
                return pkt

            def chains(sb, otp, pkt, icc_outer=False):
                """bank matmul chains for an ot pair (interleaved halves)."""
                xt = st["xt"][sb]
                xws = st["xws"][sb]

                def half(icc, p):
                    s0 = (icc * 2 + p) * 128
                    return slice(s0, s0 + 128)

                psA = [ps.tile([128, BN], f32, tag="acc", bufs=4,
                               name=f"psA{sb}_{otp}_{p}") for p in (0, 1)]
                psB = [ps.tile([128, BN], f32, tag="acc", bufs=4,
                               name=f"psB{sb}_{otp}_{p}") for p in (0, 1)]
                psC = [ps.tile([128, BN], f32,
                               tag=("p5" if icc_outer else "acc"),
                               bufs=(2 if icc_outer else 4),
                               name=f"psC{sb}_{otp}_{p}") for p in (0, 1)]
                mvs = {0: lambda n, s: xws[n][:, s],
                       1: lambda n, s: xt[(n, 0)][:, s],
                       2: lambda n, s: xt[(n, 1)][:, s]}
                pss = {0: psA, 1: psB, 2: psC}
                if icc_outer:
                    # DMA-arrival order: all three chains advance per i-chunk.
                    # pkt here is keyed (icc, n, c) with [128, 256] tiles.
                    for icc in range(IC):
                        sl = slice(icc * BN, (icc + 1) * BN)
                        for n in range(NB):
                            first = (icc == 0 and n == 0)
                            last = (icc == IC - 1 and n == NB - 1)
                            for c in range(3):
                                mv = mvs[c](n, sl)
                                for p in (0, 1):
                                    nc.tensor.matmul(
                                        pss[c][p][:],
                                        pkt[(icc, n, c)][:, p * 128:(p + 1) * 128],
                                        mv, start=first, stop=last)
                else:
                    for c in range(3):
                        for n in range(NB):
                            for icc in range(IC):
                                mv = mvs[c](n, slice(icc * BN, (icc + 1) * BN))
                                first = (n == 0 and icc == 0)
                                last = (n == NB - 1 and icc == IC - 1)
                                for p in (0, 1):
                                    nc.tensor.matmul(
                                        pss[c][p][:], pkt[(c, n)][:, half(icc, p)],
                                        mv, start=first, stop=last)
                aA = []
                for p in (0, 1):
                    a = wk.tile([128, BN], f32, tag="aA", bufs=2,
                                name=f"aA{sb}_{otp}_{p}")
                    nc.vector.tensor_copy(a[:], psA[p][:])
                    aA.append(a)
                for p in (0, 1):
                    cti = wk.tile([128, BN], f16, tag="ct", bufs=CT_BUFS,
                                  name=f"cti{sb}_{otp}_{p}")
                    nc.vector.tensor_add(cti[:], aA[p][:], psB[p][:])
                    st["ct"][(1, 2 * otp + p)] = cti
                for p in (0, 1):
                    ctr = wk.tile([128, BN], f16, tag="ct", bufs=CT_BUFS,
                                  name=f"ctr{sb}_{otp}_{p}")
                    nc.vector.tensor_sub(ctr[:], aA[p][:], psC[p][:])
                    st["ct"][(0, 2 * otp + p)] = ctr
                # incremental ctS (ctr+cti) columns for this ot pair
                if otp == 0:
                    for tb in range(SBT):
                        st["ctS"][tb] = wk.tile([128, OT * 128], f16,
                                                tag="ctS", bufs=CTS_BUFS,
                                                name=f"ctS{sb}_{tb}")
                for p in (0, 1):
                    oc = 2 * otp + p
                    for tb in range(SBT):
                        nc.vector.tensor_add(
                            st["ctS"][tb][:, oc * 128:(oc + 1) * 128],
                            st["ct"][(0, oc)][:, tb * 128:(tb + 1) * 128],
                            st["ct"][(1, oc)][:, tb * 128:(tb + 1) * 128])

            def p4(sb):
                """row sum-of-squares -> rsqrt factor (+ row inv transposes)."""
                off = sb * BN
                ssp = ps.tile([1, BN], f32, tag="sm", bufs=2, name=f"ssp{sb}")
                first = True
                for ot_ in range(OT):
                    for c in (0, 1):
                        c2 = wk.tile([128, BN], f16, tag="c2", bufs=1,
                                     name=f"c2{sb}_{ot_}_{c}")
                        nc.vector.tensor_mul(c2[:], st["ct"][(c, ot_)][:],
                                             st["ct"][(c, ot_)][:])
                        nc.tensor.matmul(ssp[:], onesD[:], c2[:], start=first,
                                         stop=(ot_ == OT - 1 and c == 1))
                        first = False
                sms = wk.tile([1, BN], f32, tag="sms", bufs=2, name=f"sms{sb}")
                nc.vector.scalar_tensor_tensor(
                    sms[:], sseps[0:1, off:off + BN], NORM_EPS, ssp[:],
                    AluOpType.mult, AluOpType.add)
                rec = wk.tile([1, BN], f32, tag="rec", bufs=2, name=f"rec{sb}")
                nc.vector.reciprocal(rec[:], sms[:])
                nc.scalar.activation(invrow[0:1, off:off + BN], rec[:], AF.Sqrt)
                for tb in range(SBT):
                    t = sb * SBT + tb
                    invp = ps.tile([128, 1], f32, tag="sm", bufs=2, name=f"invp{t}")
                    nc.tensor.transpose(invp[:],
                                        invrow[0:1, t * 128:(t + 1) * 128],
                                        one1[:])
                    invc = wk.tile([128, 1], f32, tag="invc", bufs=8,
                                   name=f"invc{t}")
                    nc.vector.tensor_copy(invc[:], invp[:])
                    st["invc"][tb] = invc

            def p5(sb, fillers):
                """final matmuls + scale + interleave + out DMA (dh-outer).

                fillers: list of closures; one issued into the DVE stream per
                (dh, tb) slot to keep next-superblock prep pipelined."""
                ct = st["ct"]
                fi_idx = 0
                for dh in range(DH):
                    oks = st.pop("oks0") if dh == 0 else oks_dma(sb, 1)
                    for tb in range(SBT):
                        t = sb * SBT + tb
                        fA = ps.tile([128, 512], f32, tag="p5", bufs=2,
                                     name=f"fA{t}_{dh}")
                        for oc in range(OT):
                            nc.tensor.matmul(
                                fA[:],
                                st["ctS"][tb][:, oc * 128:(oc + 1) * 128],
                                oks[0][:, oc * 512:(oc + 1) * 512],
                                start=(oc == 0), stop=(oc == OT - 1))
                        fa = wk.tile([128, 512], f32, tag="fa", bufs=1,
                                     name=f"fa{t}_{dh}")
                        nc.vector.tensor_copy(fa[:], fA[:])
                        fB = ps.tile([128, 512], f32, tag="p5", bufs=2,
                                     name=f"fB{t}_{dh}")
                        for oc in range(OT):
                            nc.tensor.matmul(
                                fB[:], ct[(0, oc)][:, tb * 128:(tb + 1) * 128],
                                oks[1][:, oc * 512:(oc + 1) * 512],
                                start=(oc == 0), stop=(oc == OT - 1))
                        fi = wk.tile([128, 512], f32, tag="fi", bufs=1,
                                     name=f"fi{t}_{dh}")
                        nc.vector.tensor_add(fi[:], fa[:], fB[:])
                        fC = ps.tile([128, 512], f32, tag="p5", bufs=2,
                                     name=f"fC{t}_{dh}")
                        for oc in range(OT):
                            nc.tensor.matmul(
                                fC[:], ct[(1, oc)][:, tb * 128:(tb + 1) * 128],
                                oks[2][:, oc * 512:(oc + 1) * 512],
                                start=(oc == 0), stop=(oc == OT - 1))
                        fr = wk.tile([128, 512], f32, tag="fr", bufs=1,
                                     name=f"fr{t}_{dh}")
                        nc.vector.tensor_sub(fr[:], fa[:], fC[:])
                        if fi_idx < len(fillers):
                            fillers[fi_idx]()
                            fi_idx += 1
                        oto = wk.tile([128, 1024], f32, tag="oto", bufs=1,
                                      name=f"oto{t}_{dh}")
                        ov = oto[:].rearrange("p (d c) -> p c d", c=2)
                        nc.scalar.activation(ov[:, 0], fr[:], AF.Copy,
                                             scale=st["invc"][tb][:])
                        nc.scalar.activation(ov[:, 1], fi[:], AF.Copy,
                                             scale=st["invc"][tb][:])
                        nc.scalar.dma_start(
                            outd[t * 128:(t + 1) * 128,
                                 dh * 1024:(dh + 1) * 1024], oto[:])
                for f in fillers[fi_idx:]:
                    f()

            # ================= program =================
            load_xt(0, q_outer=True)
            pkt0 = pk0_dma()
            for t in range(SBT):
                load_xu(t)
            for t in range(SBT):
                router(t)
            wbn_prep(0)
            for icc in range(IC):
                scale_icc(0, icc)
            pkt_next = []
            for sb in range(NSB):
                nxt = sb + 1
                for otp in range(OTP):
                    if nxt < NSB:
                        load_xu(SBT * nxt + otp)
                        if otp >= 1:
                            router(SBT * nxt + otp - 1)
                        if otp == OTP - 1:
                            load_xt(nxt)
                    if otp == 2:
                        st["oks0"] = oks_dma(sb, 0)
                    if sb == 0 and otp == 0:
                        pkt = pkt0
                    else:
                        pkt = pkt_next.pop(0)
                    if otp + 1 < OTP or nxt < NSB:
                        pkt_next.append(pk_dma((otp + 1) % OTP))
                    chains(sb, otp, pkt, icc_outer=(sb == 0 and otp == 0))
                fillers = []
                if nxt < NSB:
                    router(SBT * nxt + SBT - 1)
                    wbn_prep(nxt)
                    fillers = [
                        (lambda n=n, comp=comp: scale_nc(nxt, n, comp))
                        for n in range(NB) for comp in (0, 1)]
                p4(sb)
                p5(sb, fillers)

    nc.compile()
    return nc


def host_prep(inputs):
    """Build per-core in_maps from full inputs (numpy f32)."""
    f16 = np.float16
    phase = np.asarray(inputs["phase"], np.float32)
    bank_W = np.asarray(inputs["bank_W"], np.float32)
    W1 = np.asarray(inputs["W1"], np.float32)
    b1 = np.asarray(inputs["b1"], np.float32)
    W2 = np.asarray(inputs["W2"], np.float32)
    b2 = np.asarray(inputs["b2"], np.float32)
    gamma = np.asarray(inputs["gamma"], np.float32)
    out_W = np.asarray(inputs["out_W"], np.float32)
    bank_out = np.asarray(inputs["bank_out"], np.float32)

    pr, pi = phase[..., 0], phase[..., 1]
    pm = np.sqrt(pr * pr + pi * pi) + EPS
    ur, ui = (pr / pm)[:, :, None], (pi / pm)[:, :, None]
    Wr, Wi = bank_W[..., 0], bank_W[..., 1]
    Pr = Wr * ur - Wi * ui
    Pi_ = Wr * ui + Wi * ur
    KT = np.stack([Pr, Pi_ - Pr, Pr + Pi_], 1).transpose(0, 1, 3, 2)  # [n,c,i,o]
    # pk[otp, n, c, iw, (ic*2+p)*128+ow] = KT[n, c, ic*128+iw, (2*otp+p)*128+ow]
    pkarr = np.ascontiguousarray(
        KT.reshape(NB, 3, IC, 128, OTP, 2, 128).transpose(4, 0, 1, 3, 2, 5, 6)
        .reshape(OTP, NB, 3, 128, 2048).astype(f16))

    Og = out_W * gamma[None, :, None]          # scale c-dim (col index)
    Or, Oi = Og[..., 0], Og[..., 1]
    OKT = np.stack([Or, Oi - Or, Or + Oi], 0).transpose(0, 2, 1)  # [c, i, d2]
    # okk[dh, c, ow, oc*512+d2w] = OKT[c, oc*128+ow, dh*512+d2w]
    okarr = np.ascontiguousarray(
        OKT.reshape(3, OT, 128, DH, 512).transpose(3, 0, 2, 1, 4)
        .reshape(DH, 3, 128, OT * 512).astype(f16))

    # icc-major copy of otp0's pk for the DMA-paced first chain pass
    pk0arr = np.ascontiguousarray(
        pkarr[0].reshape(NB, 3, 128, IC, 256).transpose(3, 0, 1, 2, 4))

    w1tb = np.ascontiguousarray((W1 / D).T.astype(f16))             # [4, 16]
    b1cb = np.ascontiguousarray(b1[:, None].astype(np.float32))     # [16, 1]
    w2tb = np.ascontiguousarray(W2.T.astype(f16))                   # [16, 4]
    b2cb = np.ascontiguousarray(b2[:, None].astype(np.float32))     # [4, 1]
    eye16 = np.eye(128, dtype=f16)
    sel4 = np.kron(np.eye(NB), np.ones((1, 128))).astype(f16)
    one1 = np.ones((1, 1), dtype=np.float32)

    xall = bank_out.reshape(NB, B * L, D, 2).transpose(0, 3, 1, 2)
    xall = xall.astype(f16)                              # [NB, 2, B*L, D]
    # x^T: [NB, 2, IC, 128, B*L]
    xtall = np.ascontiguousarray(
        xall.transpose(0, 1, 3, 2).reshape(NB, 2, IC, 128, B * L))
    shared = dict(pk=pkarr, pk0=pk0arr, okk=okarr, w1t=w1tb, b1c=b1cb,
                  w2t=w2tb, b2c=b2cb, eye16=eye16, sel4=sel4, one1=one1)
    in_maps = []
    for k in range(NCORES):
        xud = np.ascontiguousarray(xall[:, :, k * ROWS:(k + 1) * ROWS, :])
        xtd = np.ascontiguousarray(xtall[..., k * ROWS:(k + 1) * ROWS])
        in_maps.append(dict(shared, xud=xud, xtd=xtd))
    return in_maps


_nc_cache = {}


def kernel(**inputs):
    from concourse.bass_utils import run_bass_kernel_spmd

    if "nc" not in _nc_cache:
        _nc_cache["nc"] = build_program()
    nc = _nc_cache["nc"]
    in_maps = host_prep(inputs)
    res = run_bass_kernel_spmd(nc, in_maps, core_ids=list(range(NCORES)))
    out = np.concatenate([r["out"] for r in res.results], axis=0)
    return np.ascontiguousarray(out.reshape(B, L, D, 2))


# revision 14
# speedup vs baseline: 1.0933x; 1.0933x over previous
"""Trainium2 Bass kernel for nn_AlgebraicFusion (complex bank mixer).

Algebra (per row l, bank n):
  y_n = P_n x_n            P_n = phase-folded bank_W (host precompute)
  w   = softmax(MLP(mean|x_n|^2))
  c   = sum_n w_n y_n
  out = O' (gamma * c * rsqrt(mean|c|^2 + eps))

v4 layout (per core, 1024 rows):
  - x PRE-TRANSPOSED from host ([n, comp, icc, iw, l]): chain moving
    operands come straight from DMA (no PE transposes / PSUM round-trip).
  - router scales per-column: e_n broadcast over partitions via selector
    matmul, then in-place DVE tensor_mul on the x^T tiles; xws = xr+xi.
  - 2 superblocks x 512 rows (BN=512 = one PSUM bank; 213ns matmuls hide
    107ns LDWEIGHTS); pk streamed twice.
  - chains ordered B, C, A (A last): B/C spill to f16 SBUF (bB/cC), A stays
    in PSUM; cti = psA + bB, ctr = psA - cC.  sb0-otp0 runs icc-outer in
    DMA-arrival order with A in the two "p5" banks.
  - sb1 prep (scale muls, xws adds) interleaved into p5(sb0)'s DVE stream
    as fillers; A-last ordering buys ~14us of slack for them.
  - router mag: xud rows hold [xr|xi] concat (2048) -> one DMA + one
    square-accum per (tile, bank); startup tiles split Scalar/DVE.
  - DMA queues: sync = xu(t0,t1) + xt(n0,n1) + pk0 + pk; gpsimd =
    xu(t2,t3) + xt(n2,n3) + oks; scalar = xu(t4-7) + out.

Sharding: rows (B*L = 8192) split evenly across 8 cores; weights replicated.
"""
import numpy as np

NB, B, L, D = 4, 4, 2048, 1024
EPS, NORM_EPS = 1e-8, 1e-5
NCORES = 8
ROWS = B * L // NCORES          # 1024 rows per core
NSB = 2                         # superblocks
SBT = 4                         # l-tiles per superblock
BN = SBT * 128                  # 512 rows per superblock
IC = 8                          # i-chunks (contraction 1024 = 8*128)
OT = 8                          # o-tiles
OTP = OT // 2                   # o-tile pairs
DH = 2                          # d2 halves of 512

XT_BUFS = 8
XWS_BUFS = 4
PKT_BUFS = 2
PK0_BUFS = 4
OKS_BUFS = 3
XU_BUFS = 2
CT_BUFS = 16
CTS_BUFS = 4


def build_program():
    import concourse.bacc as bacc
    import concourse.tile as tile
    import concourse.mybir as mybir

    AF = mybir.ActivationFunctionType
    from concourse.alu_op_type import AluOpType

    f16 = mybir.dt.float16
    f32 = mybir.dt.float32
    f8 = mybir.dt.float8e4

    nc = bacc.Bacc("TRN2", target_bir_lowering=False, debug=False,
                   num_devices=NCORES)

    xtd = nc.dram_tensor("xtd", [NB, 2, IC, 128, ROWS], f16,
                         kind="ExternalInput").ap()
    xud = nc.dram_tensor("xud", [NB, ROWS, 2 * D], f16,
                         kind="ExternalInput").ap()
    pk = nc.dram_tensor("pk", [OTP, NB, 3, 128, 2048], f16,
                        kind="ExternalInput").ap()
    pk0d = nc.dram_tensor("pk0", [IC, NB, 3, 128, 256], f16,
                          kind="ExternalInput").ap()
    okk = nc.dram_tensor("okk", [DH, 3, 128, OT * 512], f16,
                         kind="ExternalInput").ap()
    w1t = nc.dram_tensor("w1t", [4, 16], f16, kind="ExternalInput").ap()
    b1c = nc.dram_tensor("b1c", [16, 1], f32, kind="ExternalInput").ap()
    w2t = nc.dram_tensor("w2t", [16, 4], f16, kind="ExternalInput").ap()
    b2c = nc.dram_tensor("b2c", [4, 1], f32, kind="ExternalInput").ap()
    eye16d = nc.dram_tensor("eye16", [128, 128], f16, kind="ExternalInput").ap()
    sel4d = nc.dram_tensor("sel4", [NB, NB * 128], f16,
                           kind="ExternalInput").ap()
    one1d = nc.dram_tensor("one1", [1, 1], f32, kind="ExternalInput").ap()
    outd = nc.dram_tensor("out", [ROWS, 2 * D], f32, kind="ExternalOutput").ap()

    with tile.TileContext(nc) as tc:
        import contextlib
        with contextlib.ExitStack() as ctx:
            cst = ctx.enter_context(tc.tile_pool(name="cst", bufs=1))
            wk = ctx.enter_context(tc.tile_pool(name="wk", bufs=1))
            ps = ctx.enter_context(tc.tile_pool(name="ps", bufs=1,
                                                space="PSUM"))

            eye16 = cst.tile([128, 128], f16)
            nc.sync.dma_start(eye16[:], eye16d)
            sel4 = cst.tile([NB, NB * 128], f16)
            nc.sync.dma_start(sel4[:], sel4d)
            one1 = cst.tile([1, 1], f32)
            nc.sync.dma_start(one1[:], one1d)
            w1s = cst.tile([4, 16], f16)
            nc.sync.dma_start(w1s[:], w1t)
            b1s = cst.tile([16, 1], f32)
            nc.sync.dma_start(b1s[:], b1c)
            w2s = cst.tile([16, 4], f16)
            nc.sync.dma_start(w2s[:], w2t)
            b2s = cst.tile([4, 1], f32)
            nc.sync.dma_start(b2s[:], b2c)
            onesD = cst.tile([128, 1], f16)
            nc.vector.memset(onesD[:], 1.0 / D)
            ones4 = cst.tile([NB, 1], f16)
            nc.vector.memset(ones4[:], 1.0)
            sseps = cst.tile([1, ROWS], f16)   # s^2 per row
            invrow = cst.tile([1, ROWS], f32)  # rsqrt factor per row

            st = {"xt": {}, "xws": {}, "wbn": {}, "ct": {}, "ctS": {},
                  "invc": {}, "e4sb": {}}

            def load_xu(t, dma_eng, dve_sq=False):
                """one [128, 2D] DMA + square-accum per (tile, bank)."""
                mag32 = wk.tile([128, NB], f32, tag="mag32", bufs=2,
                                name=f"mag32_{t}")
                xcs = {}

                def issue_dma(n):
                    xc = wk.tile([128, 2 * D], f16, tag="xu", bufs=XU_BUFS,
                                 name=f"xu{t}_{n}")
                    dma_eng.dma_start(
                        xc[:], xud[n, t * 128:(t + 1) * 128, :])
                    xcs[n] = xc

                LOOK = 2
                for n in range(min(LOOK, NB)):
                    issue_dma(n)
                for n in range(NB):
                    if n + LOOK < NB:
                        issue_dma(n + LOOK)
                    junk = wk.tile([128, 2 * D], f8, tag="junk", bufs=1,
                                   name=f"junk{t}_{n}")
                    nc.scalar.activation(junk[:], xcs[n][:], AF.Square,
                                         accum_out=mag32[:, n:n + 1])
                mag = wk.tile([128, NB], f16, tag="mag", bufs=2, name=f"mag{t}")
                nc.vector.tensor_copy(mag[:], mag32[:])
                st[f"mag{t}"] = mag

            def router(t):
                """per-tile router MLP -> unnormalized scales into e4sb."""
                sb, tb = t // SBT, t % SBT
                if tb == 0:
                    st["e4sb"][sb] = wk.tile([NB, BN], f16, tag="e4sb",
                                             bufs=2, name=f"e4sb{sb}")
                e4sb = st["e4sb"][sb]
                mag = st.pop(f"mag{t}")
                mtp = ps.tile([NB, 128], f16, tag="sm", bufs=2, name=f"mtp{t}")
                nc.tensor.transpose(mtp[:], mag[:], eye16[:])
                magT = wk.tile([NB, 128], f16, tag="magT", bufs=2,
                               name=f"magT{t}")
                nc.vector.tensor_copy(magT[:], mtp[:])
                h1p = ps.tile([16, 128], f32, tag="sm", bufs=2, name=f"h1p{t}")
                nc.tensor.matmul(h1p[:], w1s[:], magT[:], start=True, stop=True)
                h1 = wk.tile([16, 128], f16, tag="h1", bufs=2, name=f"h1{t}")
                nc.scalar.activation(h1[:], h1p[:], AF.Gelu, bias=b1s[:])
                lgp = ps.tile([NB, 128], f32, tag="sm", bufs=2, name=f"lgp{t}")
                nc.tensor.matmul(lgp[:], w2s[:], h1[:], start=True, stop=True)
                nc.scalar.activation(e4sb[:, tb * 128:(tb + 1) * 128], lgp[:],
                                     AF.Exp, bias=b2s[:])
                ssump = ps.tile([1, 128], f32, tag="sm", bufs=2,
                                name=f"ssump{t}")
                nc.tensor.matmul(ssump[:], ones4[:],
                                 e4sb[:, tb * 128:(tb + 1) * 128],
                                 start=True, stop=True)
                s_sb = wk.tile([1, 128], f32, tag="s_sb", bufs=2,
                               name=f"ssb{t}")
                nc.vector.tensor_copy(s_sb[:], ssump[:])
                nc.vector.tensor_mul(sseps[0:1, t * 128:(t + 1) * 128],
                                     s_sb[:], s_sb[:])

            def load_xt(sb, q_outer=False):
                """DMA pre-transposed x^T tiles (two queues, 2 DMAs each)."""
                xt = {}
                for n in range(NB):
                    for comp in (0, 1):
                        xt[(n, comp)] = wk.tile(
                            [128, IC * BN], f16, tag="xt", bufs=XT_BUFS,
                            name=f"xt{sb}_{n}_{comp}")

                def issue(qh, n, comp):
                    i0, i1 = qh * (IC // 2), (qh + 1) * (IC // 2)
                    dst = xt[(n, comp)][:].rearrange(
                        "p (i l) -> p i l", i=IC)[:, i0:i1, :]
                    src = xtd[n, comp, i0:i1, :, sb * BN:(sb + 1) * BN] \
                        .rearrange("i p l -> p i l")
                    eng = nc.sync if n < 2 else nc.gpsimd
                    eng.dma_start(dst, src)

                if q_outer:
                    for qh in range(2):
                        for n in range(NB):
                            for comp in (0, 1):
                                issue(qh, n, comp)
                else:
                    for n in range(NB):
                        for comp in (0, 1):
                            for qh in range(2):
                                issue(qh, n, comp)
                st["xt"][sb] = xt

            def wbn_prep(sb):
                """broadcast e_n over partitions into wbn tiles; alloc xws."""
                e4sb = st["e4sb"][sb]
                wbns = {}
                for n in range(NB):
                    wp = ps.tile([128, BN], f32, tag="sm", bufs=2,
                                 name=f"wp{sb}_{n}")
                    nc.tensor.matmul(wp[:], sel4[:, n * 128:(n + 1) * 128],
                                     e4sb[:], start=True, stop=True)
                    wbn = wk.tile([128, BN], f16, tag="wbn", bufs=4,
                                  name=f"wbn{sb}_{n}")
                    nc.vector.tensor_copy(wbn[:], wp[:])
                    wbns[n] = wbn
                st["wbn"][sb] = wbns
                st["xws"][sb] = {
                    n: wk.tile([128, IC * BN], f16, tag="xws", bufs=XWS_BUFS,
                               name=f"xws{sb}_{n}") for n in range(NB)}

            def scale_icc(sb, icc):
                """scale one i-chunk of all xt tiles in place + xws slices."""
                xt, xws, wbns = st["xt"][sb], st["xws"][sb], st["wbn"][sb]
                s0, s1 = icc * BN, (icc + 1) * BN
                for n in range(NB):
                    for comp in (0, 1):
                        sl = xt[(n, comp)][:, s0:s1]
                        nc.vector.tensor_mul(sl, sl, wbns[n][:])
                    nc.vector.tensor_add(xws[n][:, s0:s1],
                                         xt[(n, 0)][:, s0:s1],
                                         xt[(n, 1)][:, s0:s1])

            def scale_nc(sb, n, comp):
                """filler: scale all i-chunks of one (n, comp) tile."""
                xt, wbns = st["xt"][sb], st["wbn"][sb]
                for icc in range(IC):
                    sl = xt[(n, comp)][:, icc * BN:(icc + 1) * BN]
                    nc.vector.tensor_mul(sl, sl, wbns[n][:])

            def xws_add(sb, n):
                """filler: xws[n] = xr' + xi' (both already scaled)."""
                xt, xws = st["xt"][sb], st["xws"][sb]
                nc.vector.tensor_add(xws[n][:], xt[(n, 0)][:], xt[(n, 1)][:])

            def pk0_dma():
                """icc-major pk tiles for the DMA-paced first otp."""
                pkt0 = {}
                for icc in range(IC):
                    for n in range(NB):
                        pt = wk.tile([128, 3 * 256], f16, tag="pk0",
                                     bufs=PK0_BUFS, name=f"pk0_{icc}_{n}")
                        dst = pt[:].rearrange("p (c w) -> p c w", c=3)
                        src = pk0d[icc, n].rearrange("c p w -> p c w")
                        nc.sync.dma_start(dst, src)
                        pkt0[(icc, n)] = pt
                return pkt0

            def oks_dma(sb, dh):
                oks = {}
                for okc in range(3):
                    ok = wk.tile([128, OT * 512], f16, tag="oks",
                                 bufs=OKS_BUFS, name=f"oks{sb}_{dh}_{okc}")
                    nc.gpsimd.dma_start(ok[:], okk[dh, okc])
                    oks[okc] = ok
                return oks

            def pk_dma(otp):
                pkt = {}
                for c in (1, 2, 0):
                    for n in range(NB):
                        pt = wk.tile([128, 2048], f16, tag="pkt",
                                     bufs=PKT_BUFS, name=f"pkt{otp}_{c}_{n}")
                        nc.sync.dma_start(pt[:], pk[otp, n, c])
                        pkt[(c, n)] = pt
                return pkt

            def chains(sb, otp, pkt, icc_outer=False):
                """bank matmul chains for an ot pair, order B, C, A."""
                xt = st["xt"][sb]
                xws = st["xws"][sb]

                psB = [ps.tile([128, BN], f32, tag="acc", bufs=4,
                               name=f"psB{sb}_{otp}_{p}") for p in (0, 1)]
                psC = [ps.tile([128, BN], f32, tag="acc", bufs=4,
                               name=f"psC{sb}_{otp}_{p}") for p in (0, 1)]
                psA = [ps.tile([128, BN], f32,
                               tag=("p5" if icc_outer else "acc"),
                               bufs=(2 if icc_outer else 4),
                               name=f"psA{sb}_{otp}_{p}") for p in (0, 1)]
                mvs = {0: lambda n, s: xws[n][:, s],
                       1: lambda n, s: xt[(n, 0)][:, s],
                       2: lambda n, s: xt[(n, 1)][:, s]}
                pss = {0: psA, 1: psB, 2: psC}

                def spill(src_ps, nm):
                    t_ = wk.tile([128, BN], f16, tag="spl", bufs=4,
                                 name=f"{nm}{sb}_{otp}")
                    nc.vector.tensor_copy(t_[:], src_ps[:])
                    return t_

                if icc_outer:
                    # DMA-arrival order; pkt keyed (icc, n), [128, 768] tiles
                    for icc in range(IC):
                        sl = slice(icc * BN, (icc + 1) * BN)
                        for n in range(NB):
                            first = (icc == 0 and n == 0)
                            last = (icc == IC - 1 and n == NB - 1)
                            for c in (1, 2, 0):
                                mv = mvs[c](n, sl)
                                for p in (0, 1):
                                    nc.tensor.matmul(
                                        pss[c][p][:],
                                        pkt[(icc, n)][:, c * 256 + p * 128:
                                                      c * 256 + (p + 1) * 128],
                                        mv, start=first, stop=last)
                    bB = [spill(psB[p], f"bB{p}_") for p in (0, 1)]
                    cC = [spill(psC[p], f"cC{p}_") for p in (0, 1)]
                else:
                    for c in (1, 2, 0):
                        for n in range(NB):
                            for icc in range(IC):
                                s0 = (icc * 2) * 128
                                mv = mvs[c](n, slice(icc * BN, (icc + 1) * BN))
                                first = (n == 0 and icc == 0)
                                last = (n == NB - 1 and icc == IC - 1)
                                for p in (0, 1):
                                    nc.tensor.matmul(
                                        pss[c][p][:],
                                        pkt[(c, n)][:, s0 + p * 128:
                                                    s0 + (p + 1) * 128],
                                        mv, start=first, stop=last)
                        if c == 1:
                            bB = [spill(psB[p], f"bB{p}_") for p in (0, 1)]
                        elif c == 2:
                            cC = [spill(psC[p], f"cC{p}_") for p in (0, 1)]
                for p in (0, 1):
                    cti = wk.tile([128, BN], f16, tag="ct", bufs=CT_BUFS,
                                  name=f"cti{sb}_{otp}_{p}")
                    nc.vector.tensor_add(cti[:], psA[p][:], bB[p][:])
                    st["ct"][(1, 2 * otp + p)] = cti
                for p in (0, 1):
                    ctr = wk.tile([128, BN], f16, tag="ct", bufs=CT_BUFS,
                                  name=f"ctr{sb}_{otp}_{p}")
                    nc.vector.tensor_sub(ctr[:], psA[p][:], cC[p][:])
                    st["ct"][(0, 2 * otp + p)] = ctr
                # incremental ctS (ctr+cti) columns for this ot pair
                if otp == 0:
                    for tb in range(SBT):
                        st["ctS"][tb] = wk.tile([128, OT * 128], f16,
                                                tag="ctS", bufs=CTS_BUFS,
                                                name=f"ctS{sb}_{tb}")
                for p in (0, 1):
                    oc = 2 * otp + p
                    for tb in range(SBT):
                        nc.vector.tensor_add(
                            st["ctS"][tb][:, oc * 128:(oc + 1) * 128],
                            st["ct"][(0, oc)][:, tb * 128:(tb + 1) * 128],
                            st["ct"][(1, oc)][:, tb * 128:(tb + 1) * 128])

            def p4(sb):
                """row sum-of-squares -> rsqrt factor (+ row inv transposes)."""
                off = sb * BN
                ssp = ps.tile([1, BN], f32, tag="sm", bufs=2, name=f"ssp{sb}")
                first = True
                for ot_ in range(OT):
                    for c in (0, 1):
                        c2 = wk.tile([128, BN], f16, tag="c2", bufs=1,
                                     name=f"c2{sb}_{ot_}_{c}")
                        nc.vector.tensor_mul(c2[:], st["ct"][(c, ot_)][:],
                                             st["ct"][(c, ot_)][:])
                        nc.tensor.matmul(ssp[:], onesD[:], c2[:], start=first,
                                         stop=(ot_ == OT - 1 and c == 1))
                        first = False
                sms = wk.tile([1, BN], f32, tag="sms", bufs=1, name=f"sms{sb}")
                nc.vector.scalar_tensor_tensor(
                    sms[:], sseps[0:1, off:off + BN], NORM_EPS, ssp[:],
                    AluOpType.mult, AluOpType.add)
                rec = wk.tile([1, BN], f32, tag="rec", bufs=1, name=f"rec{sb}")
                nc.vector.reciprocal(rec[:], sms[:])
                nc.scalar.activation(invrow[0:1, off:off + BN], rec[:], AF.Sqrt)
                for tb in range(SBT):
                    t = sb * SBT + tb
                    invp = ps.tile([128, 1], f32, tag="sm", bufs=2,
                                   name=f"invp{t}")
                    nc.tensor.transpose(invp[:],
                                        invrow[0:1, t * 128:(t + 1) * 128],
                                        one1[:])
                    invc = wk.tile([128, 1], f32, tag="invc", bufs=8,
                                   name=f"invc{t}")
                    nc.vector.tensor_copy(invc[:], invp[:])
                    st["invc"][tb] = invc

            def p5(sb, fillers):
                """final matmuls + scale + interleave + out DMA (dh-outer).

                fillers: closures issued into the DVE stream (one per
                (dh, tb) slot, remainder after) to pipeline next-sb prep."""
                ct = st["ct"]
                fi_idx = 0
                for dh in range(DH):
                    oks = st.pop("oks0") if dh == 0 else oks_dma(sb, 1)
                    for tb in range(SBT):
                        t = sb * SBT + tb
                        fA = ps.tile([128, 512], f32, tag="p5", bufs=2,
                                     name=f"fA{t}_{dh}")
                        for oc in range(OT):
                            nc.tensor.matmul(
                                fA[:],
                                st["ctS"][tb][:, oc * 128:(oc + 1) * 128],
                                oks[0][:, oc * 512:(oc + 1) * 512],
                                start=(oc == 0), stop=(oc == OT - 1))
                        fa = wk.tile([128, 512], f16, tag="fa", bufs=1,
                                     name=f"fa{t}_{dh}")
                        nc.scalar.activation(fa[:], fA[:], AF.Copy)
                        fB = ps.tile([128, 512], f32, tag="p5", bufs=2,
                                     name=f"fB{t}_{dh}")
                        for oc in range(OT):
                            nc.tensor.matmul(
                                fB[:], ct[(0, oc)][:, tb * 128:(tb + 1) * 128],
                                oks[1][:, oc * 512:(oc + 1) * 512],
                                start=(oc == 0), stop=(oc == OT - 1))
                        fi = wk.tile([128, 512], f32, tag="fi", bufs=1,
                                     name=f"fi{t}_{dh}")
                        nc.vector.tensor_add(fi[:], fa[:], fB[:])
                        fC = ps.tile([128, 512], f32, tag="p5", bufs=2,
                                     name=f"fC{t}_{dh}")
                        for oc in range(OT):
                            nc.tensor.matmul(
                                fC[:], ct[(1, oc)][:, tb * 128:(tb + 1) * 128],
                                oks[2][:, oc * 512:(oc + 1) * 512],
                                start=(oc == 0), stop=(oc == OT - 1))
                        fr = wk.tile([128, 512], f32, tag="fr", bufs=1,
                                     name=f"fr{t}_{dh}")
                        nc.vector.tensor_sub(fr[:], fa[:], fC[:])
                        if fi_idx < len(fillers):
                            fillers[fi_idx]()
                            fi_idx += 1
                        oto = wk.tile([128, 1024], f32, tag="oto", bufs=1,
                                      name=f"oto{t}_{dh}")
                        ov = oto[:].rearrange("p (d c) -> p c d", c=2)
                        nc.scalar.activation(ov[:, 0], fr[:], AF.Copy,
                                             scale=st["invc"][tb][:])
                        nc.scalar.activation(ov[:, 1], fi[:], AF.Copy,
                                             scale=st["invc"][tb][:])
                        nc.scalar.dma_start(
                            outd[t * 128:(t + 1) * 128,
                                 dh * 1024:(dh + 1) * 1024], oto[:])
                for f in fillers[fi_idx:]:
                    f()

            # ================= program =================
            load_xu(0, nc.sync)
            load_xu(1, nc.sync)
            load_xu(2, nc.gpsimd)
            load_xu(3, nc.gpsimd, dve_sq=True)
            load_xt(0, q_outer=True)
            pkt0 = pk0_dma()
            for t in range(SBT):
                router(t)
            wbn_prep(0)
            for icc in range(IC):
                scale_icc(0, icc)
            pkt_next = []
            for sb in range(NSB):
                nxt = sb + 1
                for otp in range(OTP):
                    if nxt < NSB:
                        load_xu(SBT * nxt + otp, nc.scalar)
                        if otp >= 1:
                            router(SBT * nxt + otp - 1)
                        if otp == OTP - 1:
                            load_xt(nxt)
                    if otp == 2:
                        st["oks0"] = oks_dma(sb, 0)
                    if sb == 0 and otp == 0:
                        pkt = pkt0
                    else:
                        pkt = pkt_next.pop(0)
                    if otp + 1 < OTP or nxt < NSB:
                        pkt_next.append(pk_dma((otp + 1) % OTP))
                    chains(sb, otp, pkt, icc_outer=(sb == 0 and otp == 0))
                fillers = []
                if nxt < NSB:
                    router(SBT * nxt + SBT - 1)
                    wbn_prep(nxt)
                    fillers = (
                        [(lambda n=n: scale_nc(nxt, n, 0)) for n in range(NB)]
                        + [(lambda n=n: scale_nc(nxt, n, 1)) for n in range(NB)]
                        + [(lambda n=n: xws_add(nxt, n)) for n in range(NB)])
                p4(sb)
                p5(sb, fillers)

    nc.compile()
    return nc


def host_prep(inputs):
    """Build per-core in_maps from full inputs (numpy f32)."""
    f16 = np.float16
    phase = np.asarray(inputs["phase"], np.float32)
    bank_W = np.asarray(inputs["bank_W"], np.float32)
    W1 = np.asarray(inputs["W1"], np.float32)
    b1 = np.asarray(inputs["b1"], np.float32)
    W2 = np.asarray(inputs["W2"], np.float32)
    b2 = np.asarray(inputs["b2"], np.float32)
    gamma = np.asarray(inputs["gamma"], np.float32)
    out_W = np.asarray(inputs["out_W"], np.float32)
    bank_out = np.asarray(inputs["bank_out"], np.float32)

    pr, pi = phase[..., 0], phase[..., 1]
    pm = np.sqrt(pr * pr + pi * pi) + EPS
    ur, ui = (pr / pm)[:, :, None], (pi / pm)[:, :, None]
    Wr, Wi = bank_W[..., 0], bank_W[..., 1]
    Pr = Wr * ur - Wi * ui
    Pi_ = Wr * ui + Wi * ur
    KT = np.stack([Pr, Pi_ - Pr, Pr + Pi_], 1).transpose(0, 1, 3, 2)  # [n,c,i,o]
    # pk[otp, n, c, iw, (ic*2+p)*128+ow] = KT[n, c, ic*128+iw, (2*otp+p)*128+ow]
    pkarr = np.ascontiguousarray(
        KT.reshape(NB, 3, IC, 128, OTP, 2, 128).transpose(4, 0, 1, 3, 2, 5, 6)
        .reshape(OTP, NB, 3, 128, 2048).astype(f16))
    # icc-major copy of otp0's pk for the DMA-paced first chain pass
    pk0arr = np.ascontiguousarray(
        pkarr[0].reshape(NB, 3, 128, IC, 256).transpose(3, 0, 1, 2, 4))

    Og = out_W * gamma[None, :, None]          # scale c-dim (col index)
    Or, Oi = Og[..., 0], Og[..., 1]
    OKT = np.stack([Or, Oi - Or, Or + Oi], 0).transpose(0, 2, 1)  # [c, i, d2]
    # okk[dh, c, ow, oc*512+d2w] = OKT[c, oc*128+ow, dh*512+d2w]
    okarr = np.ascontiguousarray(
        OKT.reshape(3, OT, 128, DH, 512).transpose(3, 0, 2, 1, 4)
        .reshape(DH, 3, 128, OT * 512).astype(f16))

    w1tb = np.ascontiguousarray((W1 / D).T.astype(f16))             # [4, 16]
    b1cb = np.ascontiguousarray(b1[:, None].astype(np.float32))     # [16, 1]
    w2tb = np.ascontiguousarray(W2.T.astype(f16))                   # [16, 4]
    b2cb = np.ascontiguousarray(b2[:, None].astype(np.float32))     # [4, 1]
    eye16 = np.eye(128, dtype=f16)
    sel4 = np.kron(np.eye(NB), np.ones((1, 128))).astype(f16)
    one1 = np.ones((1, 1), dtype=np.float32)

    xall = bank_out.reshape(NB, B * L, D, 2).transpose(0, 3, 1, 2)
    xall = xall.astype(f16)                              # [NB, 2, B*L, D]
    # row-major with r/i concatenated per row: [NB, B*L, 2*D]
    xuall = np.ascontiguousarray(
        xall.transpose(0, 2, 1, 3).reshape(NB, B * L, 2 * D))
    # x^T: [NB, 2, IC, 128, B*L]
    xtall = np.ascontiguousarray(
        xall.transpose(0, 1, 3, 2).reshape(NB, 2, IC, 128, B * L))
    shared = dict(pk=pkarr, pk0=pk0arr, okk=okarr, w1t=w1tb, b1c=b1cb,
                  w2t=w2tb, b2c=b2cb, eye16=eye16, sel4=sel4, one1=one1)
    in_maps = []
    for k in range(NCORES):
        xud = np.ascontiguousarray(xuall[:, k * ROWS:(k + 1) * ROWS, :])
        xtd = np.ascontiguousarray(xtall[..., k * ROWS:(k + 1) * ROWS])
        in_maps.append(dict(shared, xud=xud, xtd=xtd))
    return in_maps


_nc_cache = {}


def kernel(**inputs):
    from concourse.bass_utils import run_bass_kernel_spmd

    if "nc" not in _nc_cache:
        _nc_cache["nc"] = build_program()
    nc = _nc_cache["nc"]
    in_maps = host_prep(inputs)
    res = run_bass_kernel_spmd(nc, in_maps, core_ids=list(range(NCORES)))
    out = np.concatenate([r["out"] for r in res.results], axis=0)
    return np.ascontiguousarray(out.reshape(B, L, D, 2))
